# revision 3
# baseline (speedup 1.0000x reference)
"""Distributed DPR top-k retrieval kernel for Trainium2 (8 NeuronCores), v2.

Strategy (row-sharded docs, replicated queries, 16 virtual half-shards):
  - Host: L2-normalize queries; per core slice 62500 docs, pad to 63488 with
    ones-vectors, transpose. Dims 0-511 (chunks 0-3) upload as fp8e4m3, dims
    512-767 (chunks 4-5) as bf16 -- 1024B/doc of HBM traffic instead of 3072.
  - Device (SPMD, per core): stream doc tiles [128, {4,2}, 2048].
    Scoring is "packed": each PSUM tile [128, 512] holds two 512-doc blocks
    (A on partitions 0:64, B on 64:128) so every wide op uses all 128 lanes.
      * sims:   psum[64x2, 512] += q_chunk.T @ doc_chunk  (6 chunks/block)
      * norms:  chunks 0-2 squared on ACT -> fp8, chunk 3 on GPSIMD (idle
                engine), partition-reduced by fp8 DoubleRow ones-matmuls
                (2 chunks per stream); chunks 4-5 squared on DVE (bf16 2x),
                reduced by normal bf16 matmuls. The ones lhsT is [*, 64]
                wide so norms arrive replicated on all 128 partitions -> no
                separate broadcast matmul, full-width sqrt/reciprocal.
      * scale:  ACT sqrt (psum->sbuf evict) -> DVE reciprocal_approx_fast
                -> DVE multiply into fp16 group scores [128, tile/2].
      * top-k:  per-tile hardware max8 + max_index -> 8 candidates per
                (query, half-tile-block); pool [128, 8*n_tiles].
    Outputs: full candidate pools, vals f32 + local idx i32, [128, 248].
  - Host: merge 8 cores x 2 partition-halves of pools, drop pad ids, exact
    fp32 re-rank of the ~4k candidates per query, take top k.

Device scoring is fp8/bf16-rounded (score err ~1.5e-3) which only affects
which candidates are shipped; top-of-group margins are ~10x larger, and the
host re-rank restores exact fp32 values/ordering for everything shipped.
"""

import sys

sys.path.insert(0, "/opt/trn_rl_repo")

import numpy as np
import ml_dtypes

from concourse import bacc, mybir, tile
from concourse.bass_utils import run_bass_kernel_spmd

N_CORES = 8
B = 64
D = 768
P = 128
N_TOTAL = 500000
N_LOCAL = N_TOTAL // N_CORES  # 62500
TILE_N = 2048
N_PAD = 63488  # 31 * 2048
SUB = 512
NEG = -60000.0  # fp16-representable pad score

FP32 = mybir.dt.float32
BF16 = mybir.dt.bfloat16
FP16 = mybir.dt.float16
FP8 = mybir.dt.float8e4  # e4m3
I32 = mybir.dt.int32
U32 = mybir.dt.uint32

NC8 = 4   # chunks 0..3: fp8 stream, ACT/Pool squares, DoubleRow reduce
NC16 = 2  # chunks 4..5: bf16 stream, DVE squares (2x), normal reduce
POOL_SQUARE = True  # square fp8 chunk 3 on the idle GPSIMD engine
POOL_SPLIT = 1216   # cols of bf16 chunk 5 squared on GPSIMD (rest on DVE)
SIMS_DR = True      # fp8 DoubleRow for the sims matmuls of chunks 0-3

TILES = [(TILE_N, t * TILE_N) for t in range(N_PAD // TILE_N)]
NT = len(TILES)
POOL_W = NT * 8  # 248


def build_kernel(passes=1):
    from contextlib import ExitStack

    nc = bacc.Bacc("TRN2", debug=False, target_bir_lowering=False,
                   num_devices=N_CORES)
    q8T = nc.dram_tensor("q8T", [NC8 * P, B], FP8, kind="ExternalInput").ap()
    q16T = nc.dram_tensor("q16T", [NC16 * P, B], BF16, kind="ExternalInput").ap()
    doc8T = nc.dram_tensor("doc8T", [NC8 * P, N_PAD], FP8,
                           kind="ExternalInput").ap()
    doc16T = nc.dram_tensor("doc16T", [NC16 * P, N_PAD], BF16,
                            kind="ExternalInput").ap()
    out_vals = nc.dram_tensor("out_vals", [P, POOL_W], FP32,
                              kind="ExternalOutput").ap()
    out_idx = nc.dram_tensor("out_idx", [P, POOL_W], I32,
                             kind="ExternalOutput").ap()

    with tile.TileContext(nc) as tc, ExitStack() as ctx:
        consts = ctx.enter_context(tc.tile_pool(name="consts", bufs=1))
        docs_pool = ctx.enter_context(tc.tile_pool(name="docs", bufs=3))
        sq8_pool = ctx.enter_context(tc.tile_pool(name="sq8", bufs=2))
        s45_pool = ctx.enter_context(tc.tile_pool(name="s45", bufs=2))
        grp_pool = ctx.enter_context(tc.tile_pool(name="grp", bufs=2))
        nrm_pool = ctx.enter_context(tc.tile_pool(name="nrm", bufs=4))
        tk_pool = ctx.enter_context(tc.tile_pool(name="tk", bufs=2))
        fin_pool = ctx.enter_context(tc.tile_pool(name="fin", bufs=1))
        psum_acc = ctx.enter_context(tc.tile_pool(name="pacc", bufs=3, space="PSUM"))
        psum_nrm = ctx.enter_context(tc.tile_pool(name="pnrm", bufs=3, space="PSUM"))

        # --- constants ---
        q8 = consts.tile([P, NC8, B], FP8)
        nc.sync.dma_start(out=q8[:], in_=q8T.rearrange("(c p) b -> p c b", p=P))
        q16 = consts.tile([P, NC16, B], BF16)
        nc.sync.dma_start(out=q16[:], in_=q16T.rearrange("(c p) b -> p c b", p=P))
        ones_dr = consts.tile([P, 2, B], FP8)  # DoubleRow norm lhsT
        nc.gpsimd.memset(ones_dr[:], 1.0)
        ones_8n = consts.tile([P, B], FP8)  # normal fp8 norm lhsT (block B)
        nc.gpsimd.memset(ones_8n[:], 1.0)
        ones_nr = consts.tile([P, B], BF16)  # normal bf16 norm lhsT
        nc.gpsimd.memset(ones_nr[:], 1.0)
        # per-(tile, partition-half) index base, fp32
        base_tbl = consts.tile([P, NT], FP32)
        for t, (w_t, base) in enumerate(TILES):
            nc.gpsimd.memset(base_tbl[0:B, t:t + 1], float(base))
            nc.gpsimd.memset(base_tbl[B:P, t:t + 1], float(base + w_t // 2))

        pool_vals = fin_pool.tile([P, POOL_W], FP16)
        pool_idx_f = fin_pool.tile([P, POOL_W], FP32)

        # --- streaming phase ---
        for t, (w_t, base) in [tt for _ in range(passes) for tt in enumerate(TILES)]:
            half = w_t // 2
            d8 = docs_pool.tile([P, NC8, TILE_N], FP8, tag="d8")
            nc.sync.dma_start(
                out=d8[:, :, :w_t],
                in_=doc8T[:, base:base + w_t].rearrange("(c p) n -> p c n", p=P),
            )
            d16 = docs_pool.tile([P, NC16, TILE_N], BF16, tag="d16")
            nc.sync.dma_start(
                out=d16[:, :, :w_t],
                in_=doc16T[:, base:base + w_t].rearrange("(c p) n -> p c n", p=P),
            )

            # squares: fp8 chunks 0-2 on ACT, 3 on GPSIMD; bf16 4-5 on DVE.
            # Per-chunk ops keep engine latency low so the sqrt/recip/mul
            # chain of in-flight sub-tiles isn't blocked behind a huge op.
            sq8 = sq8_pool.tile([P, NC8, TILE_N], FP8)
            for c in range(NC8 - 1):
                nc.scalar.square(sq8[:, c, :w_t], d8[:, c, :w_t])
            c_last = NC8 - 1
            if POOL_SQUARE:
                nc.gpsimd.tensor_mul(sq8[:, c_last, :w_t], d8[:, c_last, :w_t],
                                     d8[:, c_last, :w_t])
            else:
                nc.scalar.square(sq8[:, c_last, :w_t], d8[:, c_last, :w_t])
            s45 = s45_pool.tile([P, NC16, TILE_N], BF16)
            nc.vector.tensor_mul(s45[:, 0, :w_t], d16[:, 0, :w_t],
                                 d16[:, 0, :w_t])
            sp = min(POOL_SPLIT, w_t)
            if sp:
                nc.gpsimd.tensor_mul(s45[:, 1, :sp], d16[:, 1, :sp],
                                     d16[:, 1, :sp])
            if sp < w_t:
                nc.vector.tensor_mul(s45[:, 1, sp:w_t], d16[:, 1, sp:w_t],
                                     d16[:, 1, sp:w_t])

            group = grp_pool.tile([P, TILE_N // 2], FP16)

            for s in range(half // SUB):
                sl_a = slice(s * SUB, (s + 1) * SUB)
                sl_b = slice(half + s * SUB, half + (s + 1) * SUB)

                # DoubleRow is incompatible with col-tiling (XBUS budget), so
                # only block A (output base partition 0) can use it; block B
                # runs normal fp8 matmuls (same rate as bf16: 1 col/cycle).
                acc = psum_acc.tile([P, SUB], FP32)
                for bi, sl in ((0, sl_a), (1, sl_b)):
                    po = slice(bi * B, (bi + 1) * B)
                    if SIMS_DR and bi == 0:
                        for i in range(NC8 // 2):
                            nc.tensor.matmul(
                                acc[po, :], q8[:, 2 * i:2 * i + 2, :],
                                d8[:, 2 * i:2 * i + 2, sl],
                                start=(i == 0), stop=False,
                                perf_mode=mybir.MatmulPerfMode.DoubleRow,
                            )
                    else:
                        for c in range(NC8):
                            nc.tensor.matmul(
                                acc[po, :], q8[:, c, :], d8[:, c, sl],
                                start=(c == 0), stop=False,
                            )
                    for c in range(NC16):
                        nc.tensor.matmul(
                            acc[po, :], q16[:, c, :], d16[:, c, sl],
                            start=False, stop=(c == NC16 - 1),
                        )

                nrm = psum_nrm.tile([P, SUB], FP32)
                for bi, sl in ((0, sl_a), (1, sl_b)):
                    po = slice(bi * B, (bi + 1) * B)
                    if bi == 0:
                        nc.tensor.matmul(nrm[po, :], ones_dr[:],
                                         sq8[:, 0:2, sl], start=True, stop=False,
                                         perf_mode=mybir.MatmulPerfMode.DoubleRow)
                        nc.tensor.matmul(nrm[po, :], ones_dr[:],
                                         sq8[:, 2:4, sl], start=False, stop=False,
                                         perf_mode=mybir.MatmulPerfMode.DoubleRow)
                    else:
                        for c in range(NC8):
                            nc.tensor.matmul(nrm[po, :], ones_8n[:],
                                             sq8[:, c, sl],
                                             start=(c == 0), stop=False)
                    nc.tensor.matmul(nrm[po, :], ones_nr[:], s45[:, 0, sl],
                                     start=False, stop=False)
                    nc.tensor.matmul(nrm[po, :], ones_nr[:], s45[:, 1, sl],
                                     start=False, stop=True)

                norm_sb = nrm_pool.tile([P, SUB], FP32, tag="norm")
                nc.scalar.sqrt(norm_sb[:], nrm[:])
                inv_sb = nrm_pool.tile([P, SUB], FP32, tag="inv")
                nc.vector.reciprocal_approx_fast(out=inv_sb[:], in_=norm_sb[:])
                nc.vector.tensor_mul(group[:, s * SUB:(s + 1) * SUB],
                                     acc[:], inv_sb[:])

            n_real_b = N_LOCAL - (base + half)  # real docs in block B
            if n_real_b < half:
                nc.vector.memset(group[B:P, max(n_real_b, 0):half], NEG)

            # per-tile hardware top-8 of each partition-half block
            gv = tk_pool.tile([P, 8], FP16, tag="gv")
            nc.vector.max(out=gv[:], in_=group[:, :half])
            gp = tk_pool.tile([P, 8], U32, tag="gp")
            nc.vector.max_index(out=gp[:], in_max=gv[:], in_values=group[:, :half])
            nc.vector.tensor_copy(pool_vals[:, t * 8:(t + 1) * 8], gv[:])
            gp_f = tk_pool.tile([P, 8], FP32, tag="gpf")
            nc.vector.tensor_copy(gp_f[:], gp[:])
            nc.vector.tensor_scalar_add(pool_idx_f[:, t * 8:(t + 1) * 8],
                                        gp_f[:], base_tbl[:, t:t + 1])

        # --- ship full pools; merge + re-rank on host ---
        pool_vals_f = fin_pool.tile([P, POOL_W], FP32)
        nc.vector.tensor_copy(pool_vals_f[:], pool_vals[:])
        pool_idx_i = fin_pool.tile([P, POOL_W], I32)
        nc.vector.tensor_copy(pool_idx_i[:], pool_idx_f[:])
        nc.sync.dma_start(out=out_vals, in_=pool_vals_f[:])
        nc.sync.dma_start(out=out_idx, in_=pool_idx_i[:])

    nc.compile()
    return nc


_CACHED = None


def _get_nc():
    global _CACHED
    if _CACHED is None:
        _CACHED = build_kernel()
    return _CACHED


def prepare_in_maps(q, docs):
    """q [B, D] fp32 (unnormalized), docs [N_TOTAL, D] fp32 -> per-core maps."""
    qn = q / np.linalg.norm(q, axis=1, keepdims=True)
    qT = np.ascontiguousarray(qn.T)
    q8T = qT[:NC8 * P].astype(ml_dtypes.float8_e4m3)
    q16T = qT[NC8 * P:].astype(ml_dtypes.bfloat16)
    in_maps = []
    for c in range(N_CORES):
        shard = docs[c * N_LOCAL:(c + 1) * N_LOCAL]
        pad = np.ones((N_PAD - N_LOCAL, D), dtype=np.float32)
        shT = np.concatenate([shard, pad], axis=0).T
        in_maps.append({
            "q8T": q8T, "q16T": q16T,
            "doc8T": np.ascontiguousarray(shT[:NC8 * P]).astype(
                ml_dtypes.float8_e4m3),
            "doc16T": np.ascontiguousarray(shT[NC8 * P:]).astype(
                ml_dtypes.bfloat16),
        })
    return in_maps, qn


def kernel(q_embeds, doc_embeds, k_doc):
    k = int(k_doc)
    q = np.asarray(q_embeds, dtype=np.float32)
    docs = np.asarray(doc_embeds, dtype=np.float32)
    assert q.shape == (B, D) and docs.shape == (N_TOTAL, D)

    in_maps, qn = prepare_in_maps(q, docs)
    nc = _get_nc()
    res = run_bass_kernel_spmd(nc, in_maps, list(range(N_CORES))).results

    # candidate ids: row q (block A) and q+64 (block B) of each core's pool
    cand = np.empty((B, N_CORES * 2 * POOL_W), dtype=np.int64)
    for c in range(N_CORES):
        idx = res[c]["out_idx"].astype(np.int64)  # [128, POOL_W] local ids
        valid = idx < N_LOCAL
        idx = np.where(valid, idx + c * N_LOCAL, 0)  # pad ids -> doc 0
        cand[:, (2 * c) * POOL_W:(2 * c + 1) * POOL_W] = idx[0:B]
        cand[:, (2 * c + 1) * POOL_W:(2 * c + 2) * POOL_W] = idx[B:P]

    # exact fp32 re-rank of candidates (device scoring is fp8/bf16-rounded;
    # selection margins are far larger, but final values need full fp32)
    top_vals = np.empty((B, k), dtype=np.float32)
    top_idx = np.empty((B, k), dtype=np.int32)
    for b in range(B):
        ids = np.unique(cand[b])
        cd = docs[ids]
        cdn = cd / np.linalg.norm(cd, axis=1, keepdims=True)
        vals = (cdn @ qn[b]).astype(np.float32)
        order = np.lexsort((ids, -vals))[:k]
        top_vals[b] = vals[order]
        top_idx[b] = ids[order]
    return top_vals, top_idx
